# revision 5
# baseline (speedup 1.0000x reference)
"""Fully-fused fp16 MoE expert FFN (E=8, C=2048, D=1024, F=4096), 8 TRN2 cores.

One expert per core; w1+w2 SBUF-resident fp16. vs kernel5:
- psum tiles span 2 banks [P,1024]: mm1 evicts j-pairs in one ACT pass,
  mm2 evicts dn-pairs in one DVE pass (halves group-boundary semaphore
  work on the PE queue),
- chunk-0 x / first w1 slab live in their own split tiles and the first
  four mm1 groups run k0-3 across j before k4-7, so the PE starts as
  soon as the first 1MB lands instead of waiting for the full 2MB.
"""

import numpy as np

import concourse.bass as bass
import concourse.mybir as mybir
import concourse.tile as tile
from concourse import bacc
from concourse.bass_utils import run_bass_kernel_spmd

E, C, D, F = 8, 2048, 1024, 4096
P = 128
KD = D // P  # 8
MF = F // P  # 32
CN = C // 512  # 4
CJ = 4
DN = D // 512  # 2
FJ = F // 512  # 8

F32 = mybir.dt.float32
F16 = mybir.dt.float16
GELU = mybir.ActivationFunctionType.Gelu_apprx_tanh

_CACHE = {}


def _build():
    nc = bacc.Bacc("TRN2", target_bir_lowering=False, debug=False, num_devices=E)

    xT_d = nc.dram_tensor("xT", [CN, P, KD, 512], F16, kind="ExternalInput").ap()
    w1_d = nc.dram_tensor("w1r", [FJ, P, KD, 512], F16, kind="ExternalInput").ap()
    b1_d = nc.dram_tensor("b1t", [P, MF], F32, kind="ExternalInput").ap()
    w2_d = nc.dram_tensor("w2r", [DN, P, MF, 512], F16, kind="ExternalInput").ap()
    out_d = nc.dram_tensor("out", [C, D], F32, kind="ExternalOutput").ap()

    with tile.TileContext(nc) as tc:
        with (
            tc.tile_pool(name="w1a", bufs=1) as w1a_pool,
            tc.tile_pool(name="w1b", bufs=1) as w1b_pool,
            tc.tile_pool(name="w1f", bufs=1) as w1_pool,
            tc.tile_pool(name="w2f", bufs=1) as w2_pool,
            tc.tile_pool(name="b1", bufs=1) as b1_pool,
            tc.tile_pool(name="x0a", bufs=1) as x0a_pool,
            tc.tile_pool(name="x0b", bufs=1) as x0b_pool,
            tc.tile_pool(name="xt", bufs=2) as xt_pool,
            tc.tile_pool(name="ht", bufs=1) as ht_pool,
            tc.tile_pool(name="ev", bufs=3) as ev_pool,
            tc.tile_pool(name="ps", bufs=4, space="PSUM") as ps_pool,
        ):
            b1t = b1_pool.tile([P, MF], F32)
            H = KD // 2

            # critical prefix: first 1MB lets the PE start; then the rest
            x0a = x0a_pool.tile([P, H, 512], F16)
            w10 = w1a_pool.tile([P, H, 512], F16)  # w1 jj0, k0-3
            x0b = x0b_pool.tile([P, H, 512], F16)
            w11 = w1b_pool.tile([P, H, 512], F16)  # w1 jj0, k4-7
            nc.sync.dma_start(x0a[:], xT_d[0, :, :H, :])
            nc.sync.dma_start(w10[:], w1_d[0, :, :H, :])
            nc.sync.dma_start(b1t[:], b1_d[:])
            nc.sync.dma_start(x0b[:], xT_d[0, :, H:, :])
            nc.sync.dma_start(w11[:], w1_d[0, :, H:, :])

            w1f = w1_pool.tile([P, FJ - 1, KD, 512], F16)  # jj 1..7
            w2f = w2_pool.tile([P, DN, MF, 512], F16)
            for jj in range(1, FJ):
                nc.sync.dma_start(w1f[:, jj - 1], w1_d[jj])
                if jj == 3:
                    nc.sync.dma_start(w2f[:, 0], w2_d[0])
                if jj == 5:
                    nc.sync.dma_start(w2f[:, 1], w2_d[1])

            def load_x(cn):
                t = xt_pool.tile([P, KD, 512], F16, tag="xt")
                nc.sync.dma_start(t[:], xT_d[cn])
                return t

            def w1ap(j, k):  # lhsT [P, 128] for mm1 col block j, k-tile k
                if j < 4:
                    t = w10 if k < H else w11
                    return t[:, k % H, bass.ds(j * P, P)]
                return w1f[:, j // 4 - 1, k, bass.ds((j % 4) * P, P)]

            def evict_pair(ht, ps, j):
                for u in range(2):
                    nc.scalar.activation(
                        ht[:, j + u, :],
                        ps[:, bass.ds(u * 512, 512)],
                        GELU,
                        bias=b1t[:, j + u : j + u + 1],
                    )

            xt = None
            for cn in range(CN):
                ht = ht_pool.tile([P, MF, 512], F16, tag="ht")
                if cn == 0:
                    # interleaved fast start: k0-3 across j0-3 first
                    psa = ps_pool.tile([P, 1024], F32, tag="ps")
                    psb = ps_pool.tile([P, 1024], F32, tag="ps")
                    pss = [psa, psb]
                    for j in range(4):
                        for k in range(H):
                            nc.tensor.matmul(
                                pss[j // 2][:, bass.ds((j % 2) * 512, 512)],
                                w1ap(j, k),
                                x0a[:, k, :],
                                start=(k == 0),
                                stop=False,
                                skip_group_check=True,
                            )
                    for j in range(4):
                        for k in range(H, KD):
                            nc.tensor.matmul(
                                pss[j // 2][:, bass.ds((j % 2) * 512, 512)],
                                w1ap(j, k),
                                x0b[:, k - H, :],
                                start=False,
                                stop=(k == KD - 1),
                                skip_group_check=True,
                            )
                        if k == KD - 1 and j % 2 == 1:
                            evict_pair(ht, pss[j // 2], j - 1)
                    j0 = 4
                else:
                    j0 = 0
                for j in range(j0, MF, 2):
                    ps = ps_pool.tile([P, 1024], F32, tag="ps")
                    for u in range(2):
                        for k in range(KD):
                            if cn == 0:
                                xk = x0a[:, k, :] if k < H else x0b[:, k - H, :]
                            else:
                                xk = xt[:, k, :]
                            nc.tensor.matmul(
                                ps[:, bass.ds(u * 512, 512)],
                                w1ap(j + u, k),
                                xk,
                                start=(k == 0),
                                stop=(k == KD - 1),
                                skip_group_check=True,
                            )
                    evict_pair(ht, ps, j)
                if cn + 1 < CN:
                    xt = load_x(cn + 1)
                for cj in range(CJ):
                    row = cn * 512 + cj * P
                    ps = ps_pool.tile([P, 1024], F32, tag="ps")
                    for dn in range(DN):
                        for j in range(MF):
                            nc.tensor.matmul(
                                ps[:, bass.ds(dn * 512, 512)],
                                ht[:, j, bass.ds(cj * P, P)],
                                w2f[:, dn, j, :],
                                start=(j == 0),
                                stop=(j == MF - 1),
                                skip_group_check=True,
                            )
                    ev = ev_pool.tile([P, 1024], F32, tag="ev")
                    nc.vector.tensor_copy(ev[:], ps[:])
                    nc.sync.dma_start(out_d[row : row + P, :], ev[:])

    nc.compile()
    return nc


def _get_nc():
    if "nc" not in _CACHE:
        _CACHE["nc"] = _build()
    return _CACHE["nc"]


def _in_map(x_e, w1_e, b1_e, w2_e):
    xT = (
        np.ascontiguousarray(x_e.T)
        .astype(np.float16)
        .reshape(KD, P, CN, 512)
        .transpose(2, 1, 0, 3)
    )
    w1r = (
        w1_e.astype(np.float16)
        .reshape(KD, P, FJ, 512)
        .transpose(2, 1, 0, 3)
    )
    b1t = np.ascontiguousarray(b1_e.reshape(MF, P).T)
    w2r = (
        w2_e.astype(np.float16)
        .reshape(MF, P, DN, 512)
        .transpose(2, 1, 0, 3)
    )
    return {
        "xT": np.ascontiguousarray(xT),
        "w1r": np.ascontiguousarray(w1r),
        "b1t": b1t,
        "w2r": np.ascontiguousarray(w2r),
    }


def kernel(inputs, w1, b1, w2, b2, _trace=False):
    nc = _get_nc()
    x = np.asarray(inputs, dtype=np.float32).reshape(E, C, D)
    in_maps = [
        _in_map(
            x[e],
            np.asarray(w1[e], dtype=np.float32),
            np.asarray(b1[e], dtype=np.float32),
            np.asarray(w2[e], dtype=np.float32),
        )
        for e in range(E)
    ]
    res = run_bass_kernel_spmd(nc, in_maps, list(range(E)), trace=_trace)
    out = np.stack([res.results[e]["out"] for e in range(E)])[None]
    out = out + np.asarray(b2, dtype=np.float32)[None]
    if _trace:
        _CACHE["last_results"] = res
    return out.astype(np.float32)
